# revision 1
# baseline (speedup 1.0000x reference)
"""NT-Xent (SimCLR) contrastive loss on 8 Trainium2 NeuronCores.

Strategy (data-parallel over rows of the 8192x8192 similarity matrix):
  reps = concat(emb_i, emb_j)                     # [8192, 256]
  Each core c gets reps cyclically rolled by -c*1024 rows, so its own
  1024 rows are always local rows 0..1023 -> one identical SPMD program.
  On device (per core):
    - normalize all 8192 rows (z = u / ||u||), cast bf16
    - transpose to z^T [256, 8192] via PE transposes (matmul layout)
    - sim row-block [1024, 8192] = z_own^T.T @ z^T in [128,512] psum tiles
    - exp(2*sim) + row-sum on the scalar engine (accum_out)
    - positive-pair diag extracted from psum via identity-mask reduce
  Host: denom = rowsum - e^2 (self-sim of unit rows), loss = mean(2*pos - log denom).
"""

import sys
import numpy as np

sys.path.insert(0, "/opt/trn_rl_repo")

B = 4096
D = 256
N2 = 2 * B          # 8192 rows of reps
NCORES = 8
RPC = N2 // NCORES  # 1024 rows per core
NCHUNK = 16         # column chunks of 512
CHW = 512           # chunk width
GRP = 3             # chunks per psum group (1536 wide, 3 banks)
NGRP = 6            # ceil(16/3): widths 1536*5 + 512
TEMP = 0.5
SCALE = 1.0 / TEMP  # 2.0

_CACHE = {}


def _build(repeat=1):
    """Build the SPMD Bass program once; returns (nc,).

    repeat>1 emits the whole body R times back-to-back in one NEFF — used
    only for wall-clock differencing (axon round-trip is ~100ms, so a
    single 0.1ms kernel is unmeasurable without on-device repetition).
    """
    import concourse.bass as bass
    import concourse.tile as tile
    from concourse import bacc, mybir
    from concourse.masks import make_identity

    f32 = mybir.dt.float32
    bf16 = mybir.dt.bfloat16
    Alu = mybir.AluOpType
    Act = mybir.ActivationFunctionType

    from concourse.hw_specs import get_activation_tables

    class _PinnedBacc(bacc.Bacc):
        """Pin ACT-table selection to natural_log_exp_and_others (holds
        Ln+Exp+Copy+Square+Identity) so the kernel needs one table load
        instead of thrashing between exp-only and ln-only tables."""

        def insert_act_table_loads(self):
            import bass_rust as _bass_rust
            from concourse import mybir as _mb

            has_activation = any(
                isinstance(i, _mb.InstActivation)
                for b in self.main_func.blocks
                for i in b.instructions
            )
            if not has_activation:
                return
            tables = [
                (name, funcs if name == "natural_log_exp_and_others" else set())
                for name, funcs in get_activation_tables(self.m.arch).items()
            ]
            _bass_rust.insert_act_table_loads(self, tables)

    nc = _PinnedBacc(
        "TRN2", target_bir_lowering=False, debug=False, num_devices=NCORES
    )

    reps_d = nc.dram_tensor("reps", [N2, D], f32, kind="ExternalInput").ap()
    rowsums_d = nc.dram_tensor(
        "rowsums", [128, 8 * NGRP], f32, kind="ExternalOutput"
    ).ap()
    pos_d = nc.dram_tensor("pos", [128, 8], f32, kind="ExternalOutput").ap()

    with tile.TileContext(nc) as tc:
        from contextlib import ExitStack

        with ExitStack() as ctx:
            const_pool = ctx.enter_context(tc.tile_pool(name="const", bufs=1))
            ident_bf = const_pool.tile([128, 128], bf16)
            ident_f32 = const_pool.tile([128, 128], f32)
            make_identity(nc, ident_bf[:])
            make_identity(nc, ident_f32[:])

            u_pool = ctx.enter_context(tc.tile_pool(name="u", bufs=8))
            sq_pool = ctx.enter_context(tc.tile_pool(name="sq", bufs=2))
            ss_pool = ctx.enter_context(tc.tile_pool(name="ss", bufs=4))
            z_pool = ctx.enter_context(tc.tile_pool(name="z", bufs=4))
            ptr_pool = ctx.enter_context(
                tc.tile_pool(name="ptr", bufs=2, space="PSUM")
            )
            rt_pool = ctx.enter_context(tc.tile_pool(name="rt", bufs=32))
            psb_pool = ctx.enter_context(
                tc.tile_pool(name="psb", bufs=2, space="PSUM")
            )
            exp_pool = ctx.enter_context(tc.tile_pool(name="expo", bufs=2))
            scr_pool = ctx.enter_context(tc.tile_pool(name="scr", bufs=2))
            out_pool = ctx.enter_context(tc.tile_pool(name="outp", bufs=2))

            for _rep in range(repeat):
              rowsums = out_pool.tile([128, 8 * NGRP], f32, tag="rs", name="rowsums")
              pos = out_pool.tile([128, 8], f32, tag="pos", name="pos")

              # ---------------- Phase A: build z^T [2][128, 8192] bf16 ----------
              repsT = [[None] * NCHUNK, [None] * NCHUNK]
              for n in range(NCHUNK):
                  ss = ss_pool.tile([128, 4], f32, tag="ss")
                  inv = ss_pool.tile([128, 4], f32, tag="inv")
                  lns = ss_pool.tile([128, 4], f32, tag="lns")
                  us = []
                  for tl in range(4):
                      t = 4 * n + tl
                      u = u_pool.tile([128, D], f32)
                      nc.sync.dma_start(u[:], reps_d[t * 128 : (t + 1) * 128, :])
                      us.append(u)
                      sq = sq_pool.tile([128, D], f32)
                      nc.vector.scalar_tensor_tensor(
                          out=sq[:],
                          in0=u[:],
                          scalar=1.0,
                          in1=u[:],
                          op0=Alu.bypass,
                          op1=Alu.mult,
                          accum_out=ss[:, tl : tl + 1],
                      )
                  # inv_norm = exp(-0.5 * ln(sumsq)); ln/exp share one ACT table
                  nc.scalar.activation(lns[:], ss[:], Act.Ln)
                  nc.scalar.activation(inv[:], lns[:], Act.Exp, scale=-0.5)
                  ptrs = [
                      ptr_pool.tile([128, CHW], bf16, tag="ptr", name=f"ptr{k}")
                      for k in range(2)
                  ]
                  for tl in range(4):
                      z = z_pool.tile([128, D], bf16)
                      nc.vector.tensor_scalar_mul(z[:], us[tl][:], inv[:, tl : tl + 1])
                      for k in range(2):
                          nc.tensor.transpose(
                              ptrs[k][:, tl * 128 : (tl + 1) * 128],
                              z[:, k * 128 : (k + 1) * 128],
                              ident_bf[:],
                          )
                  for k in range(2):
                      rt = rt_pool.tile([128, CHW], bf16, tag="rt")
                      nc.vector.tensor_copy(rt[:], ptrs[k][:])
                      repsT[k][n] = rt

              # ---------------- Phase B: sim row-block, exp, rowsum -------------
              for g in range(NGRP):
                  chunks = list(range(GRP * g, min(GRP * (g + 1), NCHUNK)))
                  w = CHW * len(chunks)
                  for m in range(8):
                      ps = psb_pool.tile([128, w], f32, tag="psb")
                      for k in range(2):
                          lhsT = repsT[k][m // 4][:, (m % 4) * 128 : (m % 4 + 1) * 128]
                          for ci, n in enumerate(chunks):
                              nc.tensor.matmul(
                                  ps[:, ci * CHW : (ci + 1) * CHW],
                                  lhsT,
                                  repsT[k][n][:],
                                  start=(k == 0),
                                  stop=(k == 1),
                                  skip_group_check=True,
                              )
                      # positive-pair diag: local col 4096 + m*128 + p
                      pc = 4096 + m * 128
                      if pc // CHW in chunks:
                          off = pc - chunks[0] * CHW
                          scr = scr_pool.tile([128, 128], f32, tag="scr")
                          nc.vector.scalar_tensor_tensor(
                              out=scr[:],
                              in0=ps[:, off : off + 128],
                              scalar=1.0,
                              in1=ident_f32[:],
                              op0=Alu.bypass,
                              op1=Alu.mult,
                              accum_out=pos[:, m : m + 1],
                          )
                      ex = exp_pool.tile([128, w], bf16, tag="expo")
                      nc.scalar.activation(
                          ex[:],
                          ps[:],
                          Act.Exp,
                          scale=SCALE,
                          accum_out=rowsums[:, m * NGRP + g : m * NGRP + g + 1],
                      )

              nc.sync.dma_start(rowsums_d[:], rowsums[:])
              nc.sync.dma_start(pos_d[:], pos[:])

    nc.compile()
    return nc


def _get_nc(repeat=1):
    key = ("nc", repeat)
    if key not in _CACHE:
        _CACHE[key] = _build(repeat)
    return _CACHE[key]


def kernel(emb_i: np.ndarray, emb_j: np.ndarray) -> np.ndarray:
    from concourse.bass_utils import run_bass_kernel_spmd

    nc = _get_nc()
    reps = np.concatenate(
        [np.asarray(emb_i, np.float32), np.asarray(emb_j, np.float32)], axis=0
    )
    in_maps = [
        {"reps": np.roll(reps, -c * RPC, axis=0)} for c in range(NCORES)
    ]
    res = run_bass_kernel_spmd(nc, in_maps, core_ids=list(range(NCORES)))
    return _combine(res.results)


def _combine(results) -> np.ndarray:
    # per core: rowsums [128, 8*NGRP] ([p, m*NGRP+g]), pos [128, 8] ([p, m])
    S = np.empty((NCORES, 8, 128), np.float64)   # [c, m, p] row sums
    P = np.empty((NCORES, 8, 128), np.float64)
    for c in range(NCORES):
        rs = np.asarray(results[c]["rowsums"], np.float64)  # [128, 48]
        S[c] = rs.reshape(128, 8, NGRP).sum(axis=2).T
        P[c] = np.asarray(results[c]["pos"], np.float64).T
    denom = S - np.exp(2.0)  # subtract self-similarity exp(1/T)
    loss = (2.0 * P - np.log(denom)).mean()
    return np.float32(loss)

